# revision 42
# baseline (speedup 1.0000x reference)
"""Trainium2 Bass kernel for nn_DWT_Layer: 3-level 2D db4 DWT (symmetric mode).

Input  x: (16, 3, 1024, 1024) fp32.
Output:   (16, 3, 64, 128, 128) fp32 — the flattened/truncated wavelet pyramid
          [cA3, cH3, cV3, cD3, cH2, cV2, cD2, cH1, cV1, cD1(truncated)].

Sharding: pure data parallel — the 48 (batch*channel) images are split 6 per
NeuronCore across 8 cores; no communication.

Per-core dataflow, per image, per level with input X [N, N] (fp16 in SBUF,
h-blocked into ceil(N/128) tiles):

  Both separable passes run on the tensor engine as banded matmuls against
  the folded DWT matrix M2 [R=2Np, N] (symmetric extension absorbed into the
  weights; rows [0,Np) = low-pass, [Np,2Np) = high-pass).  PE matmul computes
  lhsT^T @ rhs, so operand role-swapping yields each pass's transpose free:

    MM1:  lhsT = X[k-block][:, w-chunk],  rhs = M2^T[k-block, band]
          -> psum Yt[w-chunk] [m<=128, R]  (= (M2 @ X)^T chunk)
    MM2:  lhsT = Yt[k-block][:, i-chunk],  rhs = M2^T[k-block, band]
          -> psum Z[i-chunk] [m2<=128, R]  (= M2 @ X @ M2^T, row-major)

  Only the nonzero band of M2^T is multiplied (~2x68 rows per 128-contraction
  block); runs split where the first-writer block changes (psum start flag)
  and at psum bank boundaries.  fp16 operands (1 PE cycle/row), fp32 psum.

  Each chunk's psum = one [128,1024] tile + one [128,512] tail tile (3 banks,
  depth-3/2 rotation).  The 1536-col space packs MULTIPLE IMAGES per chunk
  for the small levels — 2 images at N=515 (2x522 cols), 3 at N=261 (3x268) —
  amortizing the per-chunk latency that dominates those levels.

  ACT/DVE/Pool copy psum->SBUF (greedy least-loaded assignment): Yt chunks
  cast to fp16 (MM2 operands), the Z quadrant aa (rows<Np, cols<Np) cast to
  fp16 as the next level's input, and detail quadrants staged fp32 in
  multi-slot tiles for batched contiguous-row DMA.  Where a detail quadrant
  starts mid-chunk (partition not in {0,32,64,96}), the copy takes whole
  partitions and the DMA slices the partition range (DMA has no
  partition-start restriction).
"""
import numpy as np

# ----------------------------------------------------------------- constants
DEC_LO = np.array([-0.010597401784997278, 0.032883011666982945,
                   0.030841381835986965, -0.18703481171888114,
                   -0.027983769416983849, 0.63088076792959036,
                   0.71484657055254153, 0.23037781330885523], dtype=np.float64)
L = 8
DEC_HI = np.array([(-1.0) ** (k + 1) * DEC_LO[L - 1 - k] for k in range(L)],
                  dtype=np.float64)

B, C, H, W = 16, 3, 1024, 1024
N_CORES = 8
IMGS_PER_CORE = 6
IMG_ELEMS = H * W

LEVEL_NS = (1024, 515, 261)
GROUP_OF = {1024: 1, 515: 2, 261: 3}  # images packed per psum chunk

# output section offsets (elements within one image's 1048576-long output)
SECT = {}
_cur = 0
for _name, _n in [("cA3", 134), ("cH3", 134), ("cV3", 134), ("cD3", 134),
                  ("cH2", 261), ("cV2", 261), ("cD2", 261),
                  ("cH1", 515), ("cV1", 515), ("cD1", 515)]:
    SECT[_name] = (_cur, _n)
    _cur += _n * _n
# cD1 truncation: keep first 469 full rows + 404 elems of row 469
CD1_FULL_ROWS = 469
CD1_PART_COLS = 404
assert SECT["cD1"][0] + CD1_FULL_ROWS * 515 + CD1_PART_COLS == IMG_ELEMS


def nprime(N):
    return (N + 5) // 2 + 1


def ext_index(j, N):
    if j < 6:
        return 5 - j
    if j < N + 6:
        return j - 6
    return 2 * N + 5 - j


def dwt_matrix(N, filt):
    Np = nprime(N)
    M = np.zeros((Np, N), dtype=np.float64)
    filtrev = filt[::-1]
    for i in range(Np):
        for t in range(L):
            M[i, ext_index(2 * i + t, N)] += filtrev[t]
    return M


def stacked_matrix(N):
    """[2Np, N]: rows [0,Np) = lo, [Np,2Np) = hi."""
    Np = nprime(N)
    M2 = np.zeros((2 * Np, N), dtype=np.float64)
    M2[:Np] = dwt_matrix(N, DEC_LO)
    M2[Np:] = dwt_matrix(N, DEC_HI)
    return M2


def build_plan(N):
    """Band-run plan for one level.

    runs[k] = [(i0, i1, start, woff)]: contiguous col-range [i0,i1) of M2^T
    for contraction block k; start = this block is the range's first writer;
    woff = col offset in the packed fp16 weights.  Ranges split where `start`
    changes.  Returns (runs, nk, R, Np, packed weights [128, total])."""
    M2 = stacked_matrix(N)
    R = M2.shape[0]
    nz = M2 != 0.0
    nk = (N + 127) // 128

    kmin = np.full(R, 10 ** 9)
    bands = {}
    for k in range(nk):
        cols = nz[:, 128 * k:128 * (k + 1)].any(axis=1)
        idx = np.nonzero(cols)[0]
        bands[k] = idx
        kmin[idx] = np.minimum(kmin[idx], k)

    total = 0
    runs = {}
    segs = []
    for k in range(nk):
        idx = bands[k]
        rr = []
        s = 0
        for e in range(1, len(idx) + 1):
            if (e == len(idx) or idx[e] != idx[e - 1] + 1
                    or (kmin[idx[e]] == k) != (kmin[idx[s]] == k)):
                i0, i1 = int(idx[s]), int(idx[e - 1]) + 1
                rr.append((i0, i1, bool(kmin[idx[s]] == k), total))
                segs.append((k, i0, i1, total))
                total += i1 - i0
                s = e
        runs[k] = rr

    packed = np.zeros((128, total), dtype=np.float16)
    for (k, i0, i1, woff) in segs:
        ksz = min(128, N - 128 * k)
        packed[:ksz, woff:woff + (i1 - i0)] = (
            M2[i0:i1, 128 * k:128 * k + ksz].T.astype(np.float16))
    return runs, nk, R, nprime(N), packed


PLAN = {N: build_plan(N) for N in LEVEL_NS}


# ------------------------------------------------------- copy-engine balance
def _emit_copy(nc, which, out, in_):
    if which == "act":
        nc.scalar.copy(out=out, in_=in_)
    elif which == "dve":
        nc.vector.tensor_copy(out=out, in_=in_)
    else:
        nc.gpsimd.tensor_copy(out=out, in_=in_)


class _Bal:
    """Greedy least-loaded assignment of copy pieces to engines.

    PSUM->SBUF copies may only use ACT/DVE (walrus: GPSIMD cannot access
    PSUM); Pool takes SBUF->SBUF casts."""
    SPEED = {"act": 1.2, "dve": 0.96, "pool": 0.72}
    OVH = {"act": 190.0, "dve": 130.0, "pool": 100.0}

    def __init__(self):
        self.t = {"act": 0.0, "dve": 0.0, "pool": 0.0}

    def charge(self, eng, width):
        self.t[eng] += width / self.SPEED[eng] + self.OVH[eng]

    def pick(self, width, exclude=()):
        cands = [e for e in ("act", "dve") if e not in exclude] \
            or ["act", "dve"]
        e = min(cands, key=lambda e: self.t[e] + width / self.SPEED[e])
        self.charge(e, width)
        return e

    def pick_cast(self, width):
        """fp32->fp16 SBUF cast: DVE runs it at 2 el/cycle, Pool at 1."""
        cost = {"dve": width / 2.0 / self.SPEED["dve"],
                "pool": width / self.SPEED["pool"]}
        e = min(("dve", "pool"), key=lambda e: self.t[e] + cost[e])
        self.t[e] += cost[e] + self.OVH[e]
        return e


class _PsumView:
    """Chunk psum = [128,1024] main tile + [128,512] tail tile: 1536 usable
    fp32 cols per chunk buffer (3 banks)."""

    def __init__(self, psA, psC):
        self.psA = psA
        self.psC = psC

    def ap(self, r0, r1, j0, j1):
        """Single piece; [j0,j1) must not straddle col 1024."""
        if j1 <= 1024:
            return self.psA[r0:r1, j0:j1]
        assert j0 >= 1024
        return self.psC[r0:r1, j0 - 1024:j1 - 1024]

    def pieces(self, j0, j1):
        """Split [j0,j1) at the 1024 tile boundary."""
        if j0 < 1024:
            yield j0, min(j1, 1024)
        if j1 > 1024:
            yield max(j0, 1024), j1

    @staticmethod
    def bank_pieces(j0, j1):
        """Split [j0,j1) at 512-col (bank) boundaries for matmul writes."""
        a = j0
        while a < j1:
            b = min((a // 512 + 1) * 512, j1)
            yield a, b
            a = b


_BUILT = None  # cached nc
_BAL = None


def build_bass(n_images=IMGS_PER_CORE, repeats=1):
    global _BAL
    _BAL = _Bal()
    import concourse.mybir as mybir
    import concourse.tile as tile
    from concourse import bacc
    from contextlib import ExitStack

    nc = bacc.Bacc("TRN2", target_bir_lowering=False, debug=False)

    xin = nc.dram_tensor("xin", (n_images, H, W), mybir.dt.float32,
                         kind="ExternalInput").ap()
    out = nc.dram_tensor("out", (n_images, IMG_ELEMS), mybir.dt.float32,
                         kind="ExternalOutput").ap()
    wdram = {}
    for N in LEVEL_NS:
        arr = PLAN[N][4]
        wdram[N] = nc.dram_tensor(f"w{N}", arr.shape, mybir.dt.float16,
                                  kind="ExternalInput").ap()

    with tile.TileContext(nc) as tc, ExitStack() as ctx:
        cpool = ctx.enter_context(tc.tile_pool(name="consts", bufs=1))
        xpool = ctx.enter_context(tc.tile_pool(name="x", bufs=1))
        ytpool = ctx.enter_context(tc.tile_pool(name="yt", bufs=1))
        psp = ctx.enter_context(tc.tile_pool(name="ps", bufs=1, space="PSUM"))
        detp = ctx.enter_context(tc.tile_pool(name="det", bufs=1))

        wsb = {}
        for N in LEVEL_NS:
            arr = PLAN[N][4]
            wsb[N] = cpool.tile(list(arr.shape), mybir.dt.float16,
                                name=f"wsb{N}")
            nc.sync.dma_start(out=wsb[N][:], in_=wdram[N])

        pools = dict(xpool=xpool, ytpool=ytpool, psp=psp, detp=detp)
        for _rep in range(repeats):
            _emit_all(nc, pools, wsb, xin, out, list(range(n_images)))

    nc.compile()
    return nc


def _emit_all(nc, pools, wsb, xin, out, imgs):
    """Emit all images: L1 per image; L2 in pairs; L3 in triples."""
    srcs1, srcs2, srcs3 = {}, {}, {}
    pend2, pend3 = [], []

    def flush2():
        nonlocal pend2
        if pend2:
            res = _emit_level_multi(nc, pools, wsb, out, 515, pend2, srcs2,
                                    ("cH2", "cV2", "cD2"), next_N=261)
            srcs3.update(res)
            pend3.extend(pend2)
            pend2 = []

    def flush3(force=False):
        nonlocal pend3
        while len(pend3) >= 3 or (force and pend3):
            grp, pend3 = pend3[:3], pend3[3:]
            _emit_level_multi(nc, pools, wsb, out, 261, grp, srcs3,
                              ("cH3", "cV3", "cD3"), next_N=None)

    # inputs are emitted one image ahead of their L1 so the in-order SP DMA
    # queue sees them before the det DMAs of L2/L3 clusters (which wait on
    # long copy chains and would head-of-line block the input transfers);
    # L3 triples are deferred past the next L1 so their latency-bound beats
    # overlap PE/copy-heavy L1 work.
    def emit_in(j):
        if j < len(imgs):
            srcs1[imgs[j]] = _emit_input(nc, pools, xin, imgs[j])

    def emit_l1(img):
        res = _emit_level_multi(nc, pools, wsb, out, 1024, [img], srcs1,
                                ("cH1", "cV1", "cD1"), next_N=515)
        srcs2.update(res)
        pend2.append(img)

    emit_in(0)
    emit_in(1)
    for j, img in enumerate(imgs):
        emit_l1(img)
        emit_in(j + 2)
        flush3()
        if len(pend2) == GROUP_OF[515]:
            flush2()
    flush2()
    flush3(force=True)


def _emit_input(nc, pools, xin, img):
    """DMA the image in 4 [128,2,1024] fp32 tiles; DVE-cast slotwise into 2
    [128,4,1024] fp16 tiles.  Returns lhsT_of(si,k,ksz,cols)."""
    import concourse.mybir as mybir
    xpool = pools["xpool"]
    N1 = 1024
    x16 = []
    for h in range(2):
        xt = xpool.tile([128, 4, N1], mybir.dt.float16, tag="x16", bufs=6,
                        name=f"x16_{img}_{h}")
        for q in range(2):
            x32 = xpool.tile([128, 2, N1], mybir.dt.float32, tag="x32",
                             bufs=3, name=f"x32_{img}_{h}_{q}")
            r0 = 512 * h + 256 * q
            src = xin[img, r0:r0 + 256, :].rearrange("(s p) w -> p s w",
                                                     p=128)
            nc.sync.dma_start(out=x32[:], in_=src)
            for s in range(2):
                eng = _BAL.pick_cast(N1)
                _emit_copy(nc, eng, xt[:, 2 * q + s, :], x32[:, s, :])
        x16.append(xt)

    def lhsT(k, ksz, c0, c1):
        return x16[k // 4][0:ksz, k % 4, c0:c1]

    return lhsT


def _det_groups(nI, Np, R):
    """Partition MM2 chunks into DMA groups with uniform partition ranges.

    Returns (lo_groups, hi_groups): each group = (c2_list, p0, p1, h0)."""
    lo, hi = [], []
    for c2 in range(nI):
        m2 = min(128, R - 128 * c2)
        lo_end = min(max(Np - 128 * c2, 0), m2)
        if lo_end > 0:
            lo.append((c2, 0, lo_end, 128 * c2))
        if 128 * c2 + m2 > Np:
            hi.append((c2, lo_end, m2, 128 * c2 + lo_end - Np))

    def group(entries):
        outg = []
        for (c2, p0, p1, h0) in entries:
            if outg and outg[-1][1] == p0 and outg[-1][2] == p1 \
                    and outg[-1][0][-1] == c2 - 1:
                outg[-1][0].append(c2)
            else:
                outg.append([[c2], p0, p1, h0])
        return [(tuple(cs), p0, p1, h0) for (cs, p0, p1, h0) in outg]

    return group(lo), group(hi)


def _emit_group_dma(nc, out, img, sec_name, Np, dt, slot0, nslots, p0, p1,
                    h0, coff=0):
    """DMA det-tile slots [slot0, slot0+nslots), partitions [p0,p1), cols
    [coff, coff+Np), to detail rows starting at h0; handles cD1
    truncation."""
    sec_base, Wd = SECT[sec_name]
    assert Wd == Np
    npart = p1 - p0
    h1 = h0 + nslots * npart
    assert 0 <= h0 and h1 <= Np, (sec_name, h0, h1)
    is_cd1 = sec_name == "cD1"

    def dma_rows(s0, ns, pa, pb, hh):
        n = ns * (pb - pa)
        dst = out[img, sec_base + hh * Wd: sec_base + (hh + n) * Wd]
        if ns > 1:
            dst = dst.rearrange("(s p w) -> p s w", p=pb - pa, w=Wd)
            nc.sync.dma_start(out=dst,
                              in_=dt[pa:pb, s0:s0 + ns, coff:coff + Wd])
        else:
            dst = dst.rearrange("(p w) -> p w", w=Wd)
            nc.sync.dma_start(out=dst, in_=dt[pa:pb, s0, coff:coff + Wd])

    if not is_cd1 or h1 <= CD1_FULL_ROWS:
        dma_rows(slot0, nslots, p0, p1, h0)
        return
    for j in range(nslots):
        hh = h0 + j * npart
        if hh >= CD1_FULL_ROWS + 1:
            break
        keep = min(npart, CD1_FULL_ROWS - hh)
        if keep > 0:
            dma_rows(slot0 + j, 1, p0, p0 + keep, hh)
        if hh <= CD1_FULL_ROWS < hh + npart:
            pp = p0 + (CD1_FULL_ROWS - hh)
            dst = out[img, sec_base + CD1_FULL_ROWS * Wd:
                      sec_base + CD1_FULL_ROWS * Wd + CD1_PART_COLS]
            nc.sync.dma_start(
                out=dst.rearrange("(p w) -> p w", w=CD1_PART_COLS),
                in_=dt[pp:pp + 1, slot0 + j, coff:coff + CD1_PART_COLS])


def _emit_level_multi(nc, pools, wsb, out, N, group, srcs, det_names,
                      next_N):
    """One DWT level for `group` (list of images packed in psum cols).

    srcs[img] = lhsT accessor fn(k, ksz, c0, c1).  Returns {img: accessor}
    for the next level (or {})."""
    import concourse.mybir as mybir
    xpool, ytpool, psp, detp = (pools["xpool"], pools["ytpool"],
                                pools["psp"], pools["detp"])
    runs, nk, R, Np, _ = PLAN[N]
    nw = (N + 127) // 128
    nI = (R + 127) // 128
    ns = len(group)
    assert ns * R <= 1536

    def mm_pass(lhsT_of, nchunks, chunk_m, sink, pass_name):
        for c in range(nchunks):
            m = chunk_m(c)
            psA = psp.tile([128, 1024], mybir.dt.float32, tag="psA", bufs=3,
                           name=f"psA_{pass_name}_{N}_{group[0]}_{c}")
            psC = None
            if ns * R > 1024:
                psC = psp.tile([128, 512], mybir.dt.float32, tag="psC",
                               bufs=2, name=f"psC_{pass_name}_{N}"
                               f"_{group[0]}_{c}")
            ps = _PsumView(psA, psC)
            # PSUM start=True zeroes the WHOLE 2KB bank (pending-zero), so
            # exactly one start per (chunk, bank): the bank's first write.
            # Later writes with start=False then either land on pending
            # bytes (auto-replace: the "fresh" ranges) or on written bytes
            # (accumulate: the band-overlap ranges) — order-independent.
            bank_started = set()
            last_mm = None
            for si in range(ns):
                base = si * R
                for k in range(nk):
                    ksz = min(128, N - 128 * k)
                    lhsT = lhsT_of(si, k, ksz, 128 * c, 128 * c + m)
                    for (i0, i1, st, woff) in runs[k]:
                        for a, b in _PsumView.bank_pieces(base + i0,
                                                          base + i1):
                            bank = a // 512
                            first = bank not in bank_started
                            bank_started.add(bank)
                            nc.tensor.matmul(
                                ps.ap(0, m, a, b),
                                lhsT,
                                wsb[N][0:ksz,
                                       woff + a - base - i0:
                                       woff + b - base - i0],
                                start=first, stop=False,
                                skip_group_check=True)
            sink(c, m, ps)

    # ---------------- MM1: Yt chunks ----------------
    yt = {img: [] for img in group}

    def mm1_sink(c, m, ps):
        used = []
        for si, img in enumerate(group):
            base = si * R
            t = ytpool.tile([128, R], mybir.dt.float16, tag=f"yt{N}",
                            bufs=ns * (nw + 1),
                            name=f"yt_{img}_{N}_{c}")
            eng = _BAL.pick(R, exclude=used)
            used.append(eng)
            for pa, pb in ps.pieces(base, base + R):
                _emit_copy(nc, eng, t[0:m, pa - base:pb - base],
                           ps.ap(0, m, pa, pb))
            yt[img].append(t)

    mm_pass(lambda si, k, ksz, c0, c1: srcs[group[si]](k, ksz, c0, c1),
            nw, lambda c: min(128, N - 128 * c), mm1_sink, "mm1")

    # ---------------- MM2: Z chunks + quadrant copies ----------------
    nkn = (Np + 127) // 128
    if next_N is not None:
        xn = {img: [xpool.tile([128, Np], mybir.dt.float16,
                               tag=f"xn{next_N}", bufs=15,
                               name=f"xn_{img}_{next_N}_{k}")
                    for k in range(nkn)]
              for img in group}
    else:
        xn = None

    lo_groups, hi_groups = _det_groups(nI, Np, R)
    lo_chunks = [c for gg in lo_groups for c in gg[0]]
    hi_chunks = [c for gg in hi_groups for c in gg[0]]
    lo_slot = {c2: i for i, c2 in enumerate(lo_chunks)}
    hi_slot = {c2: i for i, c2 in enumerate(hi_chunks)}

    dets = {}
    nbufs = {515: 1, 261: 2, 134: 3}[Np]
    for img in group:
        dets[img] = {
            "HD": detp.tile([128, len(hi_chunks), 2 * Np], mybir.dt.float32,
                            tag=f"detHD{Np}", bufs=nbufs,
                            name=f"detHD_{img}_{N}"),
        }
        if xn is not None:
            dets[img]["V"] = detp.tile([128, len(lo_chunks), Np],
                                       mybir.dt.float32, tag=f"detV{Np}",
                                       bufs=nbufs, name=f"detV_{img}_{N}")
        else:
            dets[img]["AV"] = detp.tile([128, len(lo_chunks), 2 * Np],
                                        mybir.dt.float32, tag=f"detAV{Np}",
                                        bufs=nbufs, name=f"detAV_{img}_{N}")

    def mm2_sink(c2, m2, ps):
        lo_end = min(max(Np - 128 * c2, 0), m2)
        has_lo = lo_end > 0
        has_hi = 128 * c2 + m2 > Np

        for si, img in enumerate(group):
            base = si * R
            dt = dets[img]

            jobs = []  # (width, fn(eng)) -- copy segments for this slot
            if has_lo:
                s = lo_slot[c2]
                if xn is not None:
                    jobs.append((Np, lambda e, im=img: [
                        _emit_copy(nc, e, xn[im][c2][0:lo_end, pa - base:
                                                     pb - base],
                                   ps.ap(0, lo_end, pa, pb))
                        for pa, pb in ps.pieces(base, base + Np)]))
                    jobs.append((Np, lambda e, s=s, im=img: [
                        _emit_copy(nc, e, dets[im]["V"][0:lo_end, s,
                                                        pa - base - Np:
                                                        pb - base - Np],
                                   ps.ap(0, lo_end, pa, pb))
                        for pa, pb in ps.pieces(base + Np, base + R)]))
                else:
                    # L3: cA3 and cV share one fp32 staging tile
                    jobs.append((R, lambda e, s=s, im=img: [
                        _emit_copy(nc, e, dets[im]["AV"][0:lo_end, s,
                                                         pa - base:
                                                         pb - base],
                                   ps.ap(0, lo_end, pa, pb))
                        for pa, pb in ps.pieces(base, base + R)]))
            if has_hi:
                s = hi_slot[c2]
                # cH+cD in one wide copy; whole partitions [0,m2), the DMA
                # slices the partition range
                jobs.append((R, lambda e, s=s, im=img: [
                    _emit_copy(nc, e, dets[im]["HD"][0:m2, s, pa - base:
                                                     pb - base],
                               ps.ap(0, m2, pa, pb))
                    for pa, pb in ps.pieces(base, base + R)]))
            used = []
            for w, fn in jobs:
                eng = _BAL.pick(w, exclude=used)
                used.append(eng)
                fn(eng)

            # fire group DMAs whose last chunk just completed
            for (cs, p0, p1, h0) in lo_groups:
                if cs[-1] == c2:
                    s0 = lo_slot[cs[0]]
                    if xn is not None:
                        _emit_group_dma(nc, out, img, det_names[1], Np,
                                        dt["V"], s0, len(cs), p0, p1, h0)
                    else:
                        _emit_group_dma(nc, out, img, "cA3", Np,
                                        dt["AV"], s0, len(cs), p0, p1, h0)
                        _emit_group_dma(nc, out, img, det_names[1], Np,
                                        dt["AV"], s0, len(cs), p0, p1, h0,
                                        coff=Np)
            for (cs, p0, p1, h0) in hi_groups:
                if cs[-1] == c2:
                    s0 = hi_slot[cs[0]]
                    _emit_group_dma(nc, out, img, det_names[0], Np,
                                    dt["HD"], s0, len(cs), p0, p1, h0)
                    _emit_group_dma(nc, out, img, det_names[2], Np,
                                    dt["HD"], s0, len(cs), p0, p1, h0,
                                    coff=Np)

    def mm2_lhsT(si, k, ksz, c0, c1):
        return yt[group[si]][k][0:ksz, c0:c1]

    mm_pass(mm2_lhsT, nI, lambda c: min(128, R - 128 * c), mm2_sink, "mm2")

    if xn is None:
        return {}
    res = {}
    for img in group:
        def mk(im):
            def lhsT(k, ksz, c0, c1):
                return xn[im][k][0:ksz, c0:c1]
            return lhsT
        res[img] = mk(img)
    return res


# ----------------------------------------------------------------- runner
def _get_built():
    global _BUILT
    if _BUILT is None:
        _BUILT = build_bass()
    return _BUILT


def kernel(x: np.ndarray) -> np.ndarray:
    from concourse import bass_utils

    x = np.ascontiguousarray(np.asarray(x), dtype=np.float32)
    assert x.shape == (B, C, H, W), x.shape
    nc = _get_built()

    imgs = x.reshape(B * C, H, W)
    in_maps = []
    for c in range(N_CORES):
        m = {"xin": imgs[c * IMGS_PER_CORE:(c + 1) * IMGS_PER_CORE]}
        for N in LEVEL_NS:
            m[f"w{N}"] = PLAN[N][4]
        in_maps.append(m)

    res = bass_utils.run_bass_kernel_spmd(nc, in_maps,
                                          core_ids=list(range(N_CORES)))
    outs = [res.results[c]["out"] for c in range(N_CORES)]
    flat = np.concatenate(outs, axis=0)  # [48, 1048576]
    return flat.reshape(B, C, 64, 128, 128)


# revision 43
# speedup vs baseline: 2.1880x; 2.1880x over previous
"""Trainium2 Bass kernel for nn_DWT_Layer: 3-level 2D db4 DWT (symmetric mode).

Input  x: (16, 3, 1024, 1024) fp32.
Output:   (16, 3, 64, 128, 128) fp32 — the flattened/truncated wavelet pyramid
          [cA3, cH3, cV3, cD3, cH2, cV2, cD2, cH1, cV1, cD1(truncated)].

Sharding: pure data parallel — the 48 (batch*channel) images are split 6 per
NeuronCore across 8 cores; no communication.

Per-core dataflow, per image, per level with input X [N, N] (fp16 in SBUF,
h-blocked into ceil(N/128) tiles):

  Both separable passes run on the tensor engine as banded matmuls against
  the folded DWT matrix M2 [R=2Np, N] (symmetric extension absorbed into the
  weights; rows [0,Np) = low-pass, [Np,2Np) = high-pass).  PE matmul computes
  lhsT^T @ rhs, so operand role-swapping yields each pass's transpose free:

    MM1:  lhsT = X[k-block][:, w-chunk],  rhs = M2^T[k-block, band]
          -> psum Yt[w-chunk] [m<=128, R]  (= (M2 @ X)^T chunk)
    MM2:  lhsT = Yt[k-block][:, i-chunk],  rhs = M2^T[k-block, band]
          -> psum Z[i-chunk] [m2<=128, R]  (= M2 @ X @ M2^T, row-major)

  Only the nonzero band of M2^T is multiplied (~2x68 rows per 128-contraction
  block), split at psum bank boundaries.  fp16 operands (1 PE cycle/row),
  fp32 psum.  PSUM start=True zeroes the whole 2KB bank (pending-zero), so
  each (chunk, bank) gets exactly one start=True on its first write; all
  other matmuls use start=False — fresh band ranges land on pending bytes
  (auto-replace), the 3-4 col overlaps between consecutive blocks' bands
  land on written bytes (accumulate).

  Each chunk's psum = one [128,1024] tile + one [128,512] tail tile (3 banks,
  depth-3/2 rotation).  The 1536-col space packs MULTIPLE IMAGES per chunk
  for the small levels — 2 images at N=515 (2x522 cols), 3 at N=261 (3x268) —
  amortizing the per-chunk latency that dominates those levels.

  ACT/DVE/Pool copy psum->SBUF (greedy least-loaded assignment): Yt chunks
  cast to fp16 (MM2 operands), the Z quadrant aa (rows<Np, cols<Np) cast to
  fp16 as the next level's input, and detail quadrants staged fp32 in
  multi-slot tiles for batched contiguous-row DMA.  Where a detail quadrant
  starts mid-chunk (partition not in {0,32,64,96}), the copy takes whole
  partitions and the DMA slices the partition range (DMA has no
  partition-start restriction).
"""
import numpy as np

# ----------------------------------------------------------------- constants
DEC_LO = np.array([-0.010597401784997278, 0.032883011666982945,
                   0.030841381835986965, -0.18703481171888114,
                   -0.027983769416983849, 0.63088076792959036,
                   0.71484657055254153, 0.23037781330885523], dtype=np.float64)
L = 8
DEC_HI = np.array([(-1.0) ** (k + 1) * DEC_LO[L - 1 - k] for k in range(L)],
                  dtype=np.float64)

B, C, H, W = 16, 3, 1024, 1024
N_CORES = 8
IMGS_PER_CORE = 6
IMG_ELEMS = H * W

LEVEL_NS = (1024, 515, 261)
GROUP_OF = {1024: 1, 515: 2, 261: 3}  # images packed per psum chunk

# output section offsets (elements within one image's 1048576-long output)
SECT = {}
_cur = 0
for _name, _n in [("cA3", 134), ("cH3", 134), ("cV3", 134), ("cD3", 134),
                  ("cH2", 261), ("cV2", 261), ("cD2", 261),
                  ("cH1", 515), ("cV1", 515), ("cD1", 515)]:
    SECT[_name] = (_cur, _n)
    _cur += _n * _n
# cD1 truncation: keep first 469 full rows + 404 elems of row 469
CD1_FULL_ROWS = 469
CD1_PART_COLS = 404
assert SECT["cD1"][0] + CD1_FULL_ROWS * 515 + CD1_PART_COLS == IMG_ELEMS


def nprime(N):
    return (N + 5) // 2 + 1


def ext_index(j, N):
    if j < 6:
        return 5 - j
    if j < N + 6:
        return j - 6
    return 2 * N + 5 - j


def dwt_matrix(N, filt):
    Np = nprime(N)
    M = np.zeros((Np, N), dtype=np.float64)
    filtrev = filt[::-1]
    for i in range(Np):
        for t in range(L):
            M[i, ext_index(2 * i + t, N)] += filtrev[t]
    return M


def stacked_matrix(N):
    """[2Np, N]: rows [0,Np) = lo, [Np,2Np) = hi."""
    Np = nprime(N)
    M2 = np.zeros((2 * Np, N), dtype=np.float64)
    M2[:Np] = dwt_matrix(N, DEC_LO)
    M2[Np:] = dwt_matrix(N, DEC_HI)
    return M2


def build_plan(N):
    """Band-run plan for one level.

    runs[k] = [(i0, i1, start, woff)]: contiguous col-range [i0,i1) of M2^T
    for contraction block k; start = this block is the range's first writer;
    woff = col offset in the packed fp16 weights.  Ranges split where `start`
    changes.  Returns (runs, nk, R, Np, packed weights [128, total])."""
    M2 = stacked_matrix(N)
    R = M2.shape[0]
    nz = M2 != 0.0
    nk = (N + 127) // 128

    kmin = np.full(R, 10 ** 9)
    bands = {}
    for k in range(nk):
        cols = nz[:, 128 * k:128 * (k + 1)].any(axis=1)
        idx = np.nonzero(cols)[0]
        bands[k] = idx
        kmin[idx] = np.minimum(kmin[idx], k)

    total = 0
    runs = {}
    segs = []
    for k in range(nk):
        idx = bands[k]
        rr = []
        s = 0
        for e in range(1, len(idx) + 1):
            if (e == len(idx) or idx[e] != idx[e - 1] + 1
                    or (kmin[idx[e]] == k) != (kmin[idx[s]] == k)):
                i0, i1 = int(idx[s]), int(idx[e - 1]) + 1
                rr.append((i0, i1, bool(kmin[idx[s]] == k), total))
                segs.append((k, i0, i1, total))
                total += i1 - i0
                s = e
        runs[k] = rr

    packed = np.zeros((128, total), dtype=np.float16)
    for (k, i0, i1, woff) in segs:
        ksz = min(128, N - 128 * k)
        packed[:ksz, woff:woff + (i1 - i0)] = (
            M2[i0:i1, 128 * k:128 * k + ksz].T.astype(np.float16))
    return runs, nk, R, nprime(N), packed


PLAN = {N: build_plan(N) for N in LEVEL_NS}


# ------------------------------------------------------- copy-engine balance
def _emit_copy(nc, which, out, in_):
    if which == "act":
        nc.scalar.copy(out=out, in_=in_)
    elif which == "dve":
        nc.vector.tensor_copy(out=out, in_=in_)
    else:
        nc.gpsimd.tensor_copy(out=out, in_=in_)


class _Bal:
    """Greedy least-loaded assignment of copy pieces to engines.

    PSUM->SBUF copies may only use ACT/DVE (walrus: GPSIMD cannot access
    PSUM); Pool takes SBUF->SBUF casts."""
    SPEED = {"act": 1.2, "dve": 0.96, "pool": 0.72}
    OVH = {"act": 190.0, "dve": 130.0, "pool": 100.0}

    def __init__(self):
        self.t = {"act": 0.0, "dve": 0.0, "pool": 0.0}

    def charge(self, eng, width):
        self.t[eng] += width / self.SPEED[eng] + self.OVH[eng]

    def pick(self, width, exclude=()):
        cands = [e for e in ("act", "dve") if e not in exclude] \
            or ["act", "dve"]
        e = min(cands, key=lambda e: self.t[e] + width / self.SPEED[e])
        self.charge(e, width)
        return e

    def pick_cast(self, width):
        """fp32->fp16 SBUF cast: DVE runs it at 2 el/cycle, Pool at 1."""
        cost = {"dve": width / 2.0 / self.SPEED["dve"],
                "pool": width / self.SPEED["pool"]}
        e = min(("dve", "pool"), key=lambda e: self.t[e] + cost[e])
        self.t[e] += cost[e] + self.OVH[e]
        return e


class _PsumView:
    """Chunk psum = [128,1024] main tile + [128,512] tail tile: 1536 usable
    fp32 cols per chunk buffer (3 banks)."""

    def __init__(self, psA, psC):
        self.psA = psA
        self.psC = psC

    def ap(self, r0, r1, j0, j1):
        """Single piece; [j0,j1) must not straddle col 1024."""
        if j1 <= 1024:
            return self.psA[r0:r1, j0:j1]
        assert j0 >= 1024
        return self.psC[r0:r1, j0 - 1024:j1 - 1024]

    def pieces(self, j0, j1):
        """Split [j0,j1) at the 1024 tile boundary."""
        if j0 < 1024:
            yield j0, min(j1, 1024)
        if j1 > 1024:
            yield max(j0, 1024), j1

    @staticmethod
    def bank_pieces(j0, j1):
        """Split [j0,j1) at 512-col (bank) boundaries for matmul writes."""
        a = j0
        while a < j1:
            b = min((a // 512 + 1) * 512, j1)
            yield a, b
            a = b


_BUILT = None  # cached nc
_BAL = None


def build_bass(n_images=IMGS_PER_CORE, repeats=1):
    global _BAL
    _BAL = _Bal()
    import concourse.mybir as mybir
    import concourse.tile as tile
    from concourse import bacc
    from contextlib import ExitStack

    nc = bacc.Bacc("TRN2", target_bir_lowering=False, debug=False)

    xin = nc.dram_tensor("xin", (n_images, H, W), mybir.dt.float32,
                         kind="ExternalInput").ap()
    out = nc.dram_tensor("out", (n_images, IMG_ELEMS), mybir.dt.float32,
                         kind="ExternalOutput").ap()
    wdram = {}
    for N in LEVEL_NS:
        arr = PLAN[N][4]
        wdram[N] = nc.dram_tensor(f"w{N}", arr.shape, mybir.dt.float16,
                                  kind="ExternalInput").ap()

    with tile.TileContext(nc) as tc, ExitStack() as ctx:
        cpool = ctx.enter_context(tc.tile_pool(name="consts", bufs=1))
        xpool = ctx.enter_context(tc.tile_pool(name="x", bufs=1))
        ytpool = ctx.enter_context(tc.tile_pool(name="yt", bufs=1))
        psp = ctx.enter_context(tc.tile_pool(name="ps", bufs=1, space="PSUM"))
        detp = ctx.enter_context(tc.tile_pool(name="det", bufs=1))

        wsb = {}
        for N in LEVEL_NS:
            arr = PLAN[N][4]
            wsb[N] = cpool.tile(list(arr.shape), mybir.dt.float16,
                                name=f"wsb{N}")
            nc.sync.dma_start(out=wsb[N][:], in_=wdram[N])

        pools = dict(xpool=xpool, ytpool=ytpool, psp=psp, detp=detp)
        for _rep in range(repeats):
            _emit_all(nc, pools, wsb, xin, out, list(range(n_images)))

    nc.compile()
    return nc


def _emit_all(nc, pools, wsb, xin, out, imgs):
    """Emit all images: L1 per image; L2 in pairs; L3 in triples."""
    srcs1, srcs2, srcs3 = {}, {}, {}
    pend2, pend3 = [], []

    def flush2():
        nonlocal pend2
        if pend2:
            res = _emit_level_multi(nc, pools, wsb, out, 515, pend2, srcs2,
                                    ("cH2", "cV2", "cD2"), next_N=261)
            srcs3.update(res)
            pend3.extend(pend2)
            pend2 = []

    def flush3(force=False):
        nonlocal pend3
        while len(pend3) >= 3 or (force and pend3):
            grp, pend3 = pend3[:3], pend3[3:]
            _emit_level_multi(nc, pools, wsb, out, 261, grp, srcs3,
                              ("cH3", "cV3", "cD3"), next_N=None)

    # inputs are emitted one image ahead of their L1 so the in-order SP DMA
    # queue sees them before the det DMAs of L2/L3 clusters (which wait on
    # long copy chains and would head-of-line block the input transfers);
    # L3 triples are deferred past the next L1 so their latency-bound beats
    # overlap PE/copy-heavy L1 work.
    def emit_in(j):
        if j < len(imgs):
            srcs1[imgs[j]] = _emit_input(nc, pools, xin, imgs[j])

    def emit_l1(img):
        res = _emit_level_multi(nc, pools, wsb, out, 1024, [img], srcs1,
                                ("cH1", "cV1", "cD1"), next_N=515)
        srcs2.update(res)
        pend2.append(img)

    emit_in(0)
    emit_in(1)
    for j, img in enumerate(imgs):
        emit_l1(img)
        emit_in(j + 2)
        flush3()
        if len(pend2) == GROUP_OF[515]:
            flush2()
    flush2()
    flush3(force=True)


def _emit_input(nc, pools, xin, img):
    """DMA the image in 4 [128,2,1024] fp32 tiles; DVE-cast slotwise into 2
    [128,4,1024] fp16 tiles.  Returns lhsT_of(si,k,ksz,cols)."""
    import concourse.mybir as mybir
    xpool = pools["xpool"]
    N1 = 1024
    x16 = []
    for h in range(2):
        xt = xpool.tile([128, 4, N1], mybir.dt.float16, tag="x16", bufs=6,
                        name=f"x16_{img}_{h}")
        for q in range(2):
            x32 = xpool.tile([128, 2, N1], mybir.dt.float32, tag="x32",
                             bufs=3, name=f"x32_{img}_{h}_{q}")
            r0 = 512 * h + 256 * q
            src = xin[img, r0:r0 + 256, :].rearrange("(s p) w -> p s w",
                                                     p=128)
            nc.sync.dma_start(out=x32[:], in_=src)
            for s in range(2):
                eng = _BAL.pick_cast(N1)
                _emit_copy(nc, eng, xt[:, 2 * q + s, :], x32[:, s, :])
        x16.append(xt)

    def lhsT(k, ksz, c0, c1):
        return x16[k // 4][0:ksz, k % 4, c0:c1]

    return lhsT


def _det_groups(nI, Np, R):
    """Partition MM2 chunks into DMA groups with uniform partition ranges.

    Returns (lo_groups, hi_groups): each group = (c2_list, p0, p1, h0)."""
    lo, hi = [], []
    for c2 in range(nI):
        m2 = min(128, R - 128 * c2)
        lo_end = min(max(Np - 128 * c2, 0), m2)
        if lo_end > 0:
            lo.append((c2, 0, lo_end, 128 * c2))
        if 128 * c2 + m2 > Np:
            hi.append((c2, lo_end, m2, 128 * c2 + lo_end - Np))

    def group(entries):
        outg = []
        for (c2, p0, p1, h0) in entries:
            if outg and outg[-1][1] == p0 and outg[-1][2] == p1 \
                    and outg[-1][0][-1] == c2 - 1:
                outg[-1][0].append(c2)
            else:
                outg.append([[c2], p0, p1, h0])
        return [(tuple(cs), p0, p1, h0) for (cs, p0, p1, h0) in outg]

    return group(lo), group(hi)


def _emit_group_dma(nc, out, img, sec_name, Np, dt, slot0, nslots, p0, p1,
                    h0, coff=0):
    """DMA det-tile slots [slot0, slot0+nslots), partitions [p0,p1), cols
    [coff, coff+Np), to detail rows starting at h0; handles cD1
    truncation."""
    sec_base, Wd = SECT[sec_name]
    assert Wd == Np
    npart = p1 - p0
    h1 = h0 + nslots * npart
    assert 0 <= h0 and h1 <= Np, (sec_name, h0, h1)
    is_cd1 = sec_name == "cD1"

    def dma_rows(s0, ns, pa, pb, hh):
        n = ns * (pb - pa)
        dst = out[img, sec_base + hh * Wd: sec_base + (hh + n) * Wd]
        if ns > 1:
            dst = dst.rearrange("(s p w) -> p s w", p=pb - pa, w=Wd)
            nc.sync.dma_start(out=dst,
                              in_=dt[pa:pb, s0:s0 + ns, coff:coff + Wd])
        else:
            dst = dst.rearrange("(p w) -> p w", w=Wd)
            nc.sync.dma_start(out=dst, in_=dt[pa:pb, s0, coff:coff + Wd])

    if not is_cd1 or h1 <= CD1_FULL_ROWS:
        dma_rows(slot0, nslots, p0, p1, h0)
        return
    for j in range(nslots):
        hh = h0 + j * npart
        if hh >= CD1_FULL_ROWS + 1:
            break
        keep = min(npart, CD1_FULL_ROWS - hh)
        if keep > 0:
            dma_rows(slot0 + j, 1, p0, p0 + keep, hh)
        if hh <= CD1_FULL_ROWS < hh + npart:
            pp = p0 + (CD1_FULL_ROWS - hh)
            dst = out[img, sec_base + CD1_FULL_ROWS * Wd:
                      sec_base + CD1_FULL_ROWS * Wd + CD1_PART_COLS]
            nc.sync.dma_start(
                out=dst.rearrange("(p w) -> p w", w=CD1_PART_COLS),
                in_=dt[pp:pp + 1, slot0 + j, coff:coff + CD1_PART_COLS])


def _emit_level_multi(nc, pools, wsb, out, N, group, srcs, det_names,
                      next_N):
    """One DWT level for `group` (list of images packed in psum cols).

    srcs[img] = lhsT accessor fn(k, ksz, c0, c1).  Returns {img: accessor}
    for the next level (or {})."""
    import concourse.mybir as mybir
    xpool, ytpool, psp, detp = (pools["xpool"], pools["ytpool"],
                                pools["psp"], pools["detp"])
    runs, nk, R, Np, _ = PLAN[N]
    nw = (N + 127) // 128
    nI = (R + 127) // 128
    ns = len(group)
    assert ns * R <= 1536

    def mm_pass(lhsT_of, nchunks, chunk_m, sink, pass_name):
        for c in range(nchunks):
            m = chunk_m(c)
            psA = psp.tile([128, 1024], mybir.dt.float32, tag="psA", bufs=3,
                           name=f"psA_{pass_name}_{N}_{group[0]}_{c}")
            psC = None
            if ns * R > 1024:
                psC = psp.tile([128, 512], mybir.dt.float32, tag="psC",
                               bufs=2, name=f"psC_{pass_name}_{N}"
                               f"_{group[0]}_{c}")
            ps = _PsumView(psA, psC)
            # PSUM start=True zeroes the WHOLE 2KB bank (pending-zero), so
            # exactly one start per (chunk, bank): the bank's first write.
            # Later writes with start=False then either land on pending
            # bytes (auto-replace: the "fresh" ranges) or on written bytes
            # (accumulate: the band-overlap ranges) — order-independent.
            bank_started = set()
            last_mm = None
            for si in range(ns):
                base = si * R
                for k in range(nk):
                    ksz = min(128, N - 128 * k)
                    lhsT = lhsT_of(si, k, ksz, 128 * c, 128 * c + m)
                    for (i0, i1, st, woff) in runs[k]:
                        for a, b in _PsumView.bank_pieces(base + i0,
                                                          base + i1):
                            bank = a // 512
                            first = bank not in bank_started
                            bank_started.add(bank)
                            nc.tensor.matmul(
                                ps.ap(0, m, a, b),
                                lhsT,
                                wsb[N][0:ksz,
                                       woff + a - base - i0:
                                       woff + b - base - i0],
                                start=first, stop=False,
                                skip_group_check=True)
            sink(c, m, ps)

    # ---------------- MM1: Yt chunks ----------------
    yt = {img: [] for img in group}

    def mm1_sink(c, m, ps):
        used = []
        for si, img in enumerate(group):
            base = si * R
            t = ytpool.tile([128, R], mybir.dt.float16, tag=f"yt{N}",
                            bufs=ns * (nw + 1),
                            name=f"yt_{img}_{N}_{c}")
            eng = _BAL.pick(R, exclude=used)
            used.append(eng)
            for pa, pb in ps.pieces(base, base + R):
                _emit_copy(nc, eng, t[0:m, pa - base:pb - base],
                           ps.ap(0, m, pa, pb))
            yt[img].append(t)

    mm_pass(lambda si, k, ksz, c0, c1: srcs[group[si]](k, ksz, c0, c1),
            nw, lambda c: min(128, N - 128 * c), mm1_sink, "mm1")

    # ---------------- MM2: Z chunks + quadrant copies ----------------
    nkn = (Np + 127) // 128
    if next_N is not None:
        xn = {img: [xpool.tile([128, Np], mybir.dt.float16,
                               tag=f"xn{next_N}", bufs=15,
                               name=f"xn_{img}_{next_N}_{k}")
                    for k in range(nkn)]
              for img in group}
    else:
        xn = None

    lo_groups, hi_groups = _det_groups(nI, Np, R)
    lo_chunks = [c for gg in lo_groups for c in gg[0]]
    hi_chunks = [c for gg in hi_groups for c in gg[0]]
    lo_slot = {c2: i for i, c2 in enumerate(lo_chunks)}
    hi_slot = {c2: i for i, c2 in enumerate(hi_chunks)}

    dets = {}
    nbufs = {515: 1, 261: 2, 134: 3}[Np]
    for img in group:
        dets[img] = {
            "HD": detp.tile([128, len(hi_chunks), 2 * Np], mybir.dt.float32,
                            tag=f"detHD{Np}", bufs=nbufs,
                            name=f"detHD_{img}_{N}"),
        }
        if xn is not None:
            dets[img]["V"] = detp.tile([128, len(lo_chunks), Np],
                                       mybir.dt.float32, tag=f"detV{Np}",
                                       bufs=nbufs, name=f"detV_{img}_{N}")
        else:
            dets[img]["AV"] = detp.tile([128, len(lo_chunks), 2 * Np],
                                        mybir.dt.float32, tag=f"detAV{Np}",
                                        bufs=nbufs, name=f"detAV_{img}_{N}")

    def mm2_sink(c2, m2, ps):
        lo_end = min(max(Np - 128 * c2, 0), m2)
        has_lo = lo_end > 0
        has_hi = 128 * c2 + m2 > Np

        for si, img in enumerate(group):
            base = si * R
            dt = dets[img]

            jobs = []  # (width, fn(eng)) -- copy segments for this slot
            if has_lo:
                s = lo_slot[c2]
                if xn is not None:
                    jobs.append((Np, lambda e, im=img: [
                        _emit_copy(nc, e, xn[im][c2][0:lo_end, pa - base:
                                                     pb - base],
                                   ps.ap(0, lo_end, pa, pb))
                        for pa, pb in ps.pieces(base, base + Np)]))
                    jobs.append((Np, lambda e, s=s, im=img: [
                        _emit_copy(nc, e, dets[im]["V"][0:lo_end, s,
                                                        pa - base - Np:
                                                        pb - base - Np],
                                   ps.ap(0, lo_end, pa, pb))
                        for pa, pb in ps.pieces(base + Np, base + R)]))
                else:
                    # L3: cA3 and cV share one fp32 staging tile
                    jobs.append((R, lambda e, s=s, im=img: [
                        _emit_copy(nc, e, dets[im]["AV"][0:lo_end, s,
                                                         pa - base:
                                                         pb - base],
                                   ps.ap(0, lo_end, pa, pb))
                        for pa, pb in ps.pieces(base, base + R)]))
            if has_hi:
                s = hi_slot[c2]
                # cH+cD in one wide copy; whole partitions [0,m2), the DMA
                # slices the partition range
                jobs.append((R, lambda e, s=s, im=img: [
                    _emit_copy(nc, e, dets[im]["HD"][0:m2, s, pa - base:
                                                     pb - base],
                               ps.ap(0, m2, pa, pb))
                    for pa, pb in ps.pieces(base, base + R)]))
            used = []
            for w, fn in jobs:
                eng = _BAL.pick(w, exclude=used)
                used.append(eng)
                fn(eng)

            # fire group DMAs whose last chunk just completed
            for (cs, p0, p1, h0) in lo_groups:
                if cs[-1] == c2:
                    s0 = lo_slot[cs[0]]
                    if xn is not None:
                        _emit_group_dma(nc, out, img, det_names[1], Np,
                                        dt["V"], s0, len(cs), p0, p1, h0)
                    else:
                        _emit_group_dma(nc, out, img, "cA3", Np,
                                        dt["AV"], s0, len(cs), p0, p1, h0)
                        _emit_group_dma(nc, out, img, det_names[1], Np,
                                        dt["AV"], s0, len(cs), p0, p1, h0,
                                        coff=Np)
            for (cs, p0, p1, h0) in hi_groups:
                if cs[-1] == c2:
                    s0 = hi_slot[cs[0]]
                    _emit_group_dma(nc, out, img, det_names[0], Np,
                                    dt["HD"], s0, len(cs), p0, p1, h0)
                    _emit_group_dma(nc, out, img, det_names[2], Np,
                                    dt["HD"], s0, len(cs), p0, p1, h0,
                                    coff=Np)

    def mm2_lhsT(si, k, ksz, c0, c1):
        return yt[group[si]][k][0:ksz, c0:c1]

    mm_pass(mm2_lhsT, nI, lambda c: min(128, R - 128 * c), mm2_sink, "mm2")

    if xn is None:
        return {}
    res = {}
    for img in group:
        def mk(im):
            def lhsT(k, ksz, c0, c1):
                return xn[im][k][0:ksz, c0:c1]
            return lhsT
        res[img] = mk(img)
    return res


# ----------------------------------------------------------------- runner
def _get_built():
    global _BUILT
    if _BUILT is None:
        _BUILT = build_bass()
    return _BUILT


def kernel(x: np.ndarray) -> np.ndarray:
    from concourse import bass_utils

    x = np.ascontiguousarray(np.asarray(x), dtype=np.float32)
    assert x.shape == (B, C, H, W), x.shape
    nc = _get_built()

    imgs = x.reshape(B * C, H, W)
    in_maps = []
    for c in range(N_CORES):
        m = {"xin": imgs[c * IMGS_PER_CORE:(c + 1) * IMGS_PER_CORE]}
        for N in LEVEL_NS:
            m[f"w{N}"] = PLAN[N][4]
        in_maps.append(m)

    res = bass_utils.run_bass_kernel_spmd(nc, in_maps,
                                          core_ids=list(range(N_CORES)))
    outs = [res.results[c]["out"] for c in range(N_CORES)]
    flat = np.concatenate(outs, axis=0)  # [48, 1048576]
    return flat.reshape(B, C, 64, 128, 128)
